# revision 27
# baseline (speedup 1.0000x reference)
"""Block-sparse attention (block-local) Bass kernel for 8 Trainium2 NeuronCores.

Problem: x[4, 4096, 1024] -> 4 linear projections (Q/K/V/O) + block-local
attention (block size 128, 16 heads, d_k 64), all f32.

Sharding: pure data parallel over tokens. Attention is block-local with
block size 128, so the flattened token axis [16384] splits across 8 cores
into 2048-token shards (16 blocks each) with zero cross-core communication.

Per-core kernel layout strategy:
 - x is passed host-transposed as xT [1024, 2048] so activations live in
   SBUF with d_model on partitions; Q/K projections then need no on-chip
   transposes (matmul(lhsT=W_chunk, rhs=xT_chunk)). All matmul data is fp16.
 - Q^T/K^T produced in [d_model, token] layout (what scores matmuls need),
   V in natural [token, d_model] layout (what the A@V matmul needs).
 - Per 128-token block: scores -> exp (ACT) -> row-sum (DVE) -> normalize
   (GpSimd) in [q, k] layout, then PE-transpose of A to feed A@V, whose
   [d, q] output is exactly the lhsT the final Wo projection needs.
 - The per-engine instruction order is static at runtime, so the kernel is
   software-pipelined at emission time: supertile s's attention interleaves
   with supertile s+1's projection matmuls (pure-PE filler that hides the
   cross-engine softmax chains and keeps HAM at 8/8); the last supertile's
   attention interleaves with its own V-projection chunks.
 - Host pre-layouts (weights as [m, p, c, n], broadcast biases) make every
   input DMA contiguous at 2KB/partition; all inputs are prefetched up
   front in dependency order behind a 28-matmul PE warm-up that bridges the
   ~6.5us engine preamble + first-DMA latency.
"""
import sys

if '/opt/trn_rl_repo' not in sys.path:
    sys.path.insert(0, '/opt/trn_rl_repo')

import numpy as np

import concourse.bass as bass
import concourse.mybir as mybir
import concourse.tile as tile
from concourse.vector_clock import ScopedClock
from concourse.masks import make_identity
from concourse.bass_utils import run_bass_kernel_spmd

F32 = mybir.dt.float32
F32R = mybir.dt.float32r
BF16 = mybir.dt.float16  # attention-path dtype (fp16: same PE rate, more mantissa)

D = 1024          # d_model
NH = 16           # heads
DK = 64           # head dim
BS = 128          # attention block size
N_CORES = 8
TOK = 2048        # tokens per core
ST = 512          # supertile tokens
NST = TOK // ST   # supertiles per core
SCALE = 1.0 / 8.0  # 1/sqrt(DK)

_MAX_DRAIN_WAITS = 1


class _SplitDrainTileContext(tile.TileContext):
    """The walrus in this container rejects >1 sync-wait on a NO_STRUCT
    instruction; Tile's exit drain waits on the whole global clock. Spread
    the waits across a chain of drains."""

    def _drain_and_barrier(self, tick_clock, wait_clock):
        nc = self.nc
        probe = nc.sync.drain()
        wait_clock.add_sem_waits(probe.ins, ScopedClock({None: tick_clock.global_clock}))
        si = probe.ins.sync_info
        waits = list(si.on_wait) if (si and si.on_wait) else []
        if len(waits) > _MAX_DRAIN_WAITS:
            probe.ins.sync_info = mybir.SyncInfo(
                on_wait=waits[:_MAX_DRAIN_WAITS],
                on_update=list(si.on_update) if si.on_update else [],
            )
            # Round-robin the remaining waits across all engines: each
            # engine's drains serialize, but five engines in parallel cut
            # the exit chain ~5x. The following barrier joins them.
            engs = [nc.vector, nc.scalar, nc.gpsimd, nc.tensor, nc.sync]
            for j, i in enumerate(
                    range(_MAX_DRAIN_WAITS, len(waits), _MAX_DRAIN_WAITS)):
                d = engs[j % len(engs)].drain()
                d.ins.sync_info = mybir.SyncInfo(
                    on_wait=waits[i:i + _MAX_DRAIN_WAITS], on_update=[]
                )
        nc.all_engine_barrier()
        assert self.sems is not None
        popped = nc._tile_sem_poison_stack.pop()
        assert popped is self._sem_poison
        nc.clear_and_free_semaphores(list(self.sems.allocated().values()))
        nc.all_engine_barrier()


def _split_excess_waits(nc, limit=1):
    """The nix walrus rejects instructions carrying more than `limit` sync
    waits. Hoist excess waits onto EventSemaphore instructions inserted just
    before, on the same (in-order) engine — semantics preserved."""
    n_split = 0
    for f in nc.m.functions:
        for bb in f.blocks:
            new = []
            changed = False
            for inst in bb.instructions:
                si = inst.sync_info
                waits = list(si.on_wait) if (si and si.on_wait) else []
                if len(waits) > limit:
                    excess = waits[:-limit]
                    for i in range(0, len(excess), limit):
                        ev = mybir.InstEventSemaphore(
                            name=f'I-splitw-{nc.next_id()}')
                        ev.engine = inst.engine
                        ev.sync_info = mybir.SyncInfo(
                            on_wait=excess[i:i + limit], on_update=[])
                        new.append(ev)
                        n_split += 1
                    inst.sync_info = mybir.SyncInfo(
                        on_wait=waits[-limit:],
                        on_update=list(si.on_update) if si.on_update else [])
                    changed = True
                new.append(inst)
            if changed:
                bb.instructions = new
    return n_split


def build_bass(split_waits=True):
    nc = bass.Bass('TRN2', target_bir_lowering=False, num_devices=N_CORES)

    xt_d = nc.dram_tensor('xt', [D, TOK], BF16, kind='ExternalInput')
    # weights host-pre-arranged as [m, p, c, n] so each per-m-chunk DMA is a
    # fully contiguous [128, 1024] fp16 copy (2KB/partition descriptors).
    wq_d = nc.dram_tensor('wq', [8, 128, D], BF16, kind='ExternalInput')
    wk_d = nc.dram_tensor('wk', [8, 128, D], BF16, kind='ExternalInput')
    wv_d = nc.dram_tensor('wv', [8, 128, D], BF16, kind='ExternalInput')
    wo_d = nc.dram_tensor('wo', [8, 128, D], BF16, kind='ExternalInput')
    bq_d = nc.dram_tensor('bq', [128, 8], F32, kind='ExternalInput')
    bk_d = nc.dram_tensor('bk', [128, 8], F32, kind='ExternalInput')
    bv_d = nc.dram_tensor('bv', [128, D], F32, kind='ExternalInput')
    bo_d = nc.dram_tensor('bo', [1, D], BF16, kind='ExternalInput')
    bobc_d = nc.dram_tensor('bobc', [128, D], F32, kind='ExternalInput')
    ones_d = nc.dram_tensor('ones', [1, 128], BF16, kind='ExternalInput')
    out_d = nc.dram_tensor('out', [TOK, D], F32, kind='ExternalOutput')

    with _SplitDrainTileContext(nc) as tc:
        _build_body(nc, tc, xt_d, wq_d, wk_d, wv_d, wo_d,
                    bq_d, bk_d, bv_d, bo_d, bobc_d, ones_d, out_d)
    if split_waits:
        # CoreSim chokes on the inserted EventSemaphores; only split for HW.
        _split_excess_waits(nc, limit=1)
    return nc


def _build_body(nc, tc, xt_d, wq_d, wk_d, wv_d, wo_d, bq_d, bk_d, bv_d, bo_d, bobc_d, ones_d, out_d):
    from contextlib import ExitStack
    with ExitStack() as ctx:
        _build_pools_and_body(nc, tc, ctx, xt_d, wq_d, wk_d, wv_d, wo_d,
                              bq_d, bk_d, bv_d, bo_d, bobc_d, ones_d, out_d)


def _build_pools_and_body(nc, tc, ctx, xt_d, wq_d, wk_d, wv_d, wo_d,
                          bq_d, bk_d, bv_d, bo_d, bobc_d, ones_d, out_d):
    AF = mybir.ActivationFunctionType
    OP = mybir.AluOpType
    AX = mybir.AxisListType

    wpool = ctx.enter_context(tc.tile_pool(name='w', bufs=1))
    cpool = ctx.enter_context(tc.tile_pool(name='c', bufs=1))
    xpool = ctx.enter_context(tc.tile_pool(name='x', bufs=1))
    qkv = ctx.enter_context(tc.tile_pool(name='qkv', bufs=1))
    apool = ctx.enter_context(tc.tile_pool(name='a', bufs=2))
    opool = ctx.enter_context(tc.tile_pool(name='o', bufs=2))
    otpool = ctx.enter_context(tc.tile_pool(name='ot', bufs=2))

    # One shared 4-deep ring of 2KB PSUM slots serves both the projection
    # matmuls and the attention scores (they occupy disjoint phases); pat
    # holds transposed-A (fp16), pav the two A@V accumulators.
    pp = ctx.enter_context(tc.tile_pool(name='pp', bufs=4, space='PSUM'))
    pat = ctx.enter_context(tc.tile_pool(name='pat', bufs=2, space='PSUM'))
    pav = ctx.enter_context(tc.tile_pool(name='pav', bufs=1, space='PSUM'))

    # ---- constants / weights ----
    # DMA queue order IS the arrival order: xt0 and wq first (they gate the
    # first real matmuls), then the small constants, then wk/wv/wo, then the
    # remaining supertiles' activations. All transfers are contiguous
    # (host-side pre-layout), so the queue runs at full HBM rate.
    xt_tiles = [None] * NST
    xt_tiles[0] = xpool.tile([128, 8, ST], BF16, name='xt', tag='xt0')
    nc.sync.dma_start(
        out=xt_tiles[0],
        in_=xt_d.ap()[:, 0:ST].rearrange('(c p) t -> p c t', p=128),
    )

    # w_sb layout [p, m, c, n]: the per-m DMA writes one contiguous
    # 2KB/partition block; the (c, m) matmul lhsT slice is [p, 128] strided.
    w_sb = {}
    for nm in ('q', 'k', 'v', 'o'):
        w_sb[nm] = wpool.tile([128, 8, 8, 128], BF16, name=f'w{nm}',
                              tag=f'w{nm}')

    def _load_w(nm, wd):
        for m in range(8):
            nc.sync.dma_start(
                out=w_sb[nm][:, m, :, :],
                in_=wd.ap()[m].rearrange('p (c n) -> p c n', c=8))

    _load_w('q', wq_d)

    bq_sb = cpool.tile([128, 8], F32, name='bq', tag='bq')
    nc.sync.dma_start(out=bq_sb, in_=bq_d.ap())
    bk_sb = cpool.tile([128, 8], F32, name='bk', tag='bk')
    nc.sync.dma_start(out=bk_sb, in_=bk_d.ap())
    bv_bc = cpool.tile([128, D], F32, name='bvbc', tag='bvbc')
    nc.sync.dma_start(out=bv_bc, in_=bv_d.ap())
    bo_sb = cpool.tile([1, D], BF16, name='bo', tag='bo')
    nc.sync.dma_start(out=bo_sb, in_=bo_d.ap())
    bo_bc = cpool.tile([128, D], F32, name='bobc', tag='bobc')
    nc.sync.dma_start(out=bo_bc, in_=bobc_d.ap())
    ones_sb = cpool.tile([1, 128], BF16, name='ones', tag='ones')
    nc.sync.dma_start(out=ones_sb, in_=ones_d.ap())

    ident = cpool.tile([128, 128], BF16, name='ident', tag='ident')
    make_identity(nc, ident)

    # PE warm-up: HAM un-throttles only after ~3.4us of sustained activity,
    # and the DMA pipe needs ~13us before xt0+wq have landed. Dummy matmuls
    # rotate through the shared PSUM ring so they issue back-to-back.
    warm_sb = cpool.tile([128, 512], BF16, name='warm', tag='warm')
    nc.vector.memset(warm_sb, 0.5)
    for _ in range(28):
        ps_warm = pp.tile([128, ST], F32, name='ps', tag='ps')
        nc.tensor.matmul(ps_warm, lhsT=warm_sb[:, 0:128], rhs=warm_sb,
                         start=True, stop=True)

    _load_w('k', wk_d)
    _load_w('v', wv_d)
    _load_w('o', wo_d)

    for s in range(1, NST):
        xt_tiles[s] = xpool.tile([128, 8, ST], BF16, name='xt', tag=f'xt{s}')
        nc.sync.dma_start(
            out=xt_tiles[s],
            in_=xt_d.ap()[:, s * ST:(s + 1) * ST].rearrange(
                '(c p) t -> p c t', p=128),
        )

    def r_(ap):
        return ap

    def r_(ap):
        return ap

    # ---- cross-supertile software pipeline ----
    # Supertile s's attention is interleaved instruction-by-instruction with
    # supertile s+1's projections: the projection matmuls are pure-PE filler
    # that hides the attention's cross-engine softmax chains AND keeps the
    # PE dense enough that HAM never re-throttles. The per-engine order is
    # static at runtime, so this must be done at emission time.
    proj_state = {}

    def emit_proj_unit(s, idx):
        """idx 0..7: Q chunk m; 8..15: K chunk m; 16..23: V chunk."""
        st = proj_state[s]
        xt_sb, qt_sb, kt_sb, v_sb = st['xt'], st['qt'], st['kt'], st['v']
        if idx < 8:
            m = idx
            ps = pp.tile([128, ST], F32, name='ps', tag='ps')
            for c in range(8):
                nc.tensor.matmul(ps, lhsT=r_(w_sb['q'][:, m, c, :]),
                                 rhs=r_(xt_sb[:, c, :]),
                                 start=(c == 0), stop=(c == 7))
            nc.vector.tensor_scalar(out=qt_sb[:, m, :], in0=ps,
                                    scalar1=bq_sb[:, m:m + 1], scalar2=SCALE,
                                    op0=OP.add, op1=OP.mult)
        elif idx < 16:
            m = idx - 8
            ps = pp.tile([128, ST], F32, name='ps', tag='ps')
            for c in range(8):
                nc.tensor.matmul(ps, lhsT=r_(w_sb['k'][:, m, c, :]),
                                 rhs=r_(xt_sb[:, c, :]),
                                 start=(c == 0), stop=(c == 7))
            nc.vector.tensor_scalar(out=kt_sb[:, m, :], in0=ps,
                                    scalar1=bk_sb[:, m:m + 1], scalar2=None,
                                    op0=OP.add)
        else:
            tch, nh2 = divmod(idx - 16, 2)
            ps = pp.tile([128, ST], F32, name='ps', tag='ps')
            for c in range(8):
                nc.tensor.matmul(
                    ps, lhsT=r_(xt_sb[:, c, tch * 128:(tch + 1) * 128]),
                    rhs=r_(w_sb['v'][:, nh2 * 4:(nh2 + 1) * 4, c, :]),
                    start=(c == 0), stop=(c == 7))
            nc.vector.tensor_tensor(
                out=v_sb[:, tch, nh2 * 512:(nh2 + 1) * 512], in0=ps,
                in1=bv_bc[:, nh2 * 512:(nh2 + 1) * 512], op=OP.add)

    def open_proj(s):
        proj_state[s] = {
            'xt': xt_tiles[s],
            'qt': qkv.tile([128, 8, ST], BF16, name='qt', tag='qt', bufs=2),
            'kt': qkv.tile([128, 8, ST], BF16, name='kt', tag='kt', bufs=2),
            'v': qkv.tile([128, 4, D], BF16, name='v', tag='v', bufs=2),
        }

    def _groups():
        for g in range(4):
            parity = g % 2
            base = (g // 2) * 8
            yield ([base + parity + 2 * i for i in range(4)], parity * 64)

    attn_state = {}

    def S_phase(s, b4):
        # Heads grouped by parity: every scores matmul in a group reads
        # Q^T/K^T at the SAME partition offset; mixing offsets across
        # matmuls that write one PSUM bank wedges the device.
        st = proj_state[s]
        qt_sb, kt_sb = st['qt'], st['kt']
        t0 = b4 * 128
        e_tiles = []
        for heads, off in _groups():
            ps_sc = pp.tile([128, 4, 128], F32, name='ps_sc', tag='ps')
            for i, hh in enumerate(heads):
                m = hh // 2
                nc.tensor.matmul(
                    ps_sc[:, i, :],
                    lhsT=qt_sb[off:off + 64, m, t0:t0 + 128],
                    rhs=kt_sb[off:off + 64, m, t0:t0 + 128],
                    start=True, stop=True)
            e_sb = apool.tile([128, 4, 128], BF16, name='e', tag='e', bufs=8)
            nc.scalar.activation(e_sb, ps_sc, AF.Exp)
            stat = apool.tile([128, 8], F32, name='stat', tag='stat', bufs=8)
            nc.vector.reduce_sum(out=stat[:, 0:4], in_=e_sb, axis=AX.X)
            nc.vector.reciprocal(stat[:, 4:8], stat[:, 0:4])
            nc.gpsimd.tensor_tensor(
                out=e_sb, in0=e_sb,
                in1=stat[:, 4:8].to_broadcast((128, 4, 128)),
                op=OP.mult)
            e_tiles.append(e_sb)
        attn_state[(s, b4)] = e_tiles

    def T_phase(s, b4):
        e_tiles = attn_state.pop((s, b4))
        v_sb = proj_state[s]['v']
        ps_av0 = pav.tile([128, 4, 128], F32, name='ps_av0')
        ps_av1 = pav.tile([128, 4, 128], F32, name='ps_av1')
        ot_sb = otpool.tile([128, 8, 128], BF16, name='ot')
        for g, (heads, off) in enumerate(_groups()):
            e_sb = e_tiles[g]
            ps_at = pat.tile([128, 4, 128], BF16, name='ps_at')
            for i in range(4):
                nc.tensor.transpose(ps_at[:, i, :], e_sb[:, i, :], ident)
            at_sb = apool.tile([128, 4, 128], BF16, name='at', tag='at',
                               bufs=4)
            nc.scalar.copy(at_sb, ps_at)
            for i, hh in enumerate(heads):
                g2 = hh // 2
                ps_av = ps_av0 if g2 < 4 else ps_av1
                nc.tensor.matmul(
                    ps_av[off:off + 64, g2 % 4, :],
                    lhsT=v_sb[:, b4, hh * 64:(hh + 1) * 64],
                    rhs=at_sb[:, i, :],
                    start=True, stop=True)
            if g == 1:
                nc.vector.tensor_copy(ot_sb[:, 0:4, :], ps_av0)
            elif g == 3:
                nc.vector.tensor_copy(ot_sb[:, 4:8, :], ps_av1)
        return ot_sb

    def O_phase(s, b4, ot_sb, bias_mm=False):
        # bias via the DVE eviction add normally (saves a K=1 PE matmul per
        # half); via the ones-matmul for the unpaired tail blocks where the
        # DVE is the jammed engine and the PE has slack.
        t0 = b4 * 128
        for nh2 in range(2):
            ps = pp.tile([128, ST], F32, name='ps', tag='ps')
            for c in range(8):
                nc.tensor.matmul(
                    ps, lhsT=r_(ot_sb[:, c, :]),
                    rhs=r_(w_sb['o'][:, nh2 * 4:(nh2 + 1) * 4, c, :]),
                    start=(c == 0), stop=(not bias_mm and c == 7))
            out_sb = opool.tile([128, 512], F32, name='outsb')
            if bias_mm:
                nc.tensor.matmul(ps, lhsT=r_(ones_sb),
                                 rhs=r_(bo_sb[:, nh2 * 512:(nh2 + 1) * 512]),
                                 start=False, stop=True)
                nc.vector.tensor_copy(out_sb, ps)
            else:
                nc.vector.tensor_tensor(
                    out=out_sb, in0=ps,
                    in1=bo_bc[:, nh2 * 512:(nh2 + 1) * 512], op=OP.add)
            nc.sync.dma_start(
                out=out_d.ap()[s * ST + t0: s * ST + t0 + 128,
                               nh2 * 512:(nh2 + 1) * 512],
                in_=out_sb)

    def emit_attn_unit(s, u):
        """12 units per supertile: interleavable S/T/O phases of 4 blocks,
        ordered so every T sits a full S-phase behind its scores."""
        seq = [('S', 0), ('S', 1), ('T', 0), ('O', 0), ('S', 2), ('T', 1),
               ('O', 1), ('S', 3), ('T', 2), ('O', 2), ('T', 3), ('O', 3)]
        kind, b = seq[u]
        if kind == 'S':
            S_phase(s, b)
        elif kind == 'T':
            attn_state[('ot', s, b)] = T_phase(s, b)
        else:
            O_phase(s, b, attn_state.pop(('ot', s, b)))

    # ST0 projections run alone (behind the warmup); thereafter supertile
    # s's projection chunks interleave with supertile s-1's 12 attention
    # units. For the last supertile only Q/K interleave there; its V chunks
    # are saved to cover the last supertile's own early attention blocks, so
    # the unpaired engine-jammed tail shrinks from 4 blocks to ~2.
    open_proj(0)
    for idx in range(24):
        emit_proj_unit(0, idx)
    LAST = NST - 1
    for s in range(1, NST):
        open_proj(s)
        n_front = 16 if s == LAST else 24
        counts = [2] * 12
        if s == LAST:
            counts = [2, 2, 1, 1, 2, 1, 1, 2, 1, 1, 1, 1]
        pi = 0
        for u in range(12):
            emit_attn_unit(s - 1, u)
            for _ in range(counts[u]):
                if pi < n_front:
                    emit_proj_unit(s, pi); pi += 1
        while pi < n_front:
            emit_proj_unit(s, pi); pi += 1
    # Final supertile: V-projection chunks (16..23) interleaved with its
    # attention. T(3,b) needs v[:, b, :] = V units 16+2b, 17+2b.
    s = LAST
    S_phase(s, 0)
    emit_proj_unit(s, 16); emit_proj_unit(s, 17)
    S_phase(s, 1)
    emit_proj_unit(s, 18); emit_proj_unit(s, 19)
    ot0 = T_phase(s, 0)
    emit_proj_unit(s, 20)
    O_phase(s, 0, ot0)
    emit_proj_unit(s, 21)
    S_phase(s, 2)
    emit_proj_unit(s, 22)
    ot1 = T_phase(s, 1)
    emit_proj_unit(s, 23)
    O_phase(s, 1, ot1)
    S_phase(s, 3)
    ot2 = T_phase(s, 2)
    O_phase(s, 2, ot2, bias_mm=True)
    ot3 = T_phase(s, 3)
    O_phase(s, 3, ot3, bias_mm=True)


_NC_CACHE = []


def _get_nc():
    if not _NC_CACHE:
        _NC_CACHE.append(build_bass())
    return _NC_CACHE[0]


def shard_inputs(x, Wq, bq, Wk, bk, Wv, bv, Wo, bo):
    x = np.asarray(x, dtype=np.float32)
    B, S, _ = x.shape
    xf = np.ascontiguousarray(x.reshape(B * S, D))
    assert B * S == N_CORES * TOK

    def warr(W):
        # [m, p, c, n] = W[c*128+p, m*128+n]: every per-m DMA is contiguous.
        W = np.asarray(W, dtype=np.float16)
        return np.ascontiguousarray(
            W.reshape(8, 128, 8, 128).transpose(2, 1, 0, 3).reshape(8, 128, D))

    shared = {
        'wq': warr(Wq),
        'wk': warr(Wk),
        'wv': warr(Wv),
        'wo': warr(Wo),
        'bq': np.ascontiguousarray(np.asarray(bq, dtype=np.float32).reshape(8, 128).T),
        'bk': np.ascontiguousarray(np.asarray(bk, dtype=np.float32).reshape(8, 128).T),
        'bv': np.ascontiguousarray(
            np.broadcast_to(np.asarray(bv, dtype=np.float32).reshape(1, D),
                            (128, D))),
        'bo': np.ascontiguousarray(np.asarray(bo, dtype=np.float16).reshape(1, D)),
        'bobc': np.ascontiguousarray(
            np.broadcast_to(np.asarray(bo, dtype=np.float32).reshape(1, D),
                            (128, D))),
        'ones': np.ones((1, 128), dtype=np.float16),
    }
    in_maps = []
    for c in range(N_CORES):
        xt = np.ascontiguousarray(xf[c * TOK:(c + 1) * TOK, :].T.astype(np.float16))
        in_maps.append({'xt': xt, **shared})
    return (B, S), in_maps


def run(inputs, **spmd_kwargs):
    (B, S), in_maps = shard_inputs(**inputs)
    nc = _get_nc()
    res = run_bass_kernel_spmd(nc, in_maps, list(range(N_CORES)), **spmd_kwargs)
    out = np.concatenate([res.results[c]['out'] for c in range(N_CORES)], axis=0)
    return out.reshape(B, S, D), res


def kernel(x, Wq, bq, Wk, bk, Wv, bv, Wo, bo):
    out, _ = run(dict(x=x, Wq=Wq, bq=bq, Wk=Wk, bk=bk,
                      Wv=Wv, bv=bv, Wo=Wo, bo=bo))
    return out



# revision 28
# speedup vs baseline: 1.1689x; 1.1689x over previous
"""Block-sparse attention (block-local) Bass kernel for 8 Trainium2 NeuronCores.

Problem: x[4, 4096, 1024] -> 4 linear projections (Q/K/V/O) + block-local
attention (block size 128, 16 heads, d_k 64), all f32.

Sharding: pure data parallel over tokens. Attention is block-local with
block size 128, so the flattened token axis [16384] splits across 8 cores
into 2048-token shards (16 blocks each) with zero cross-core communication.

Per-core kernel layout strategy:
 - x is passed host-transposed as xT [1024, 2048] so activations live in
   SBUF with d_model on partitions; Q/K projections then need no on-chip
   transposes (matmul(lhsT=W_chunk, rhs=xT_chunk)). All matmul data is fp16.
 - Q^T/K^T produced in [d_model, token] layout (what scores matmuls need),
   V in natural [token, d_model] layout (what the A@V matmul needs).
 - Per 128-token block: scores -> exp (ACT) -> row-sum (DVE) -> normalize
   (GpSimd) in [q, k] layout, then PE-transpose of A to feed A@V, whose
   [d, q] output is exactly the lhsT the final Wo projection needs.
 - The per-engine instruction order is static at runtime, so the kernel is
   software-pipelined at emission time: supertile s's attention interleaves
   with supertile s+1's projection matmuls (pure-PE filler that hides the
   cross-engine softmax chains and keeps HAM at 8/8); the last supertile's
   attention interleaves with its own V-projection chunks.
 - Host pre-layouts (weights as [m, p, c, n], broadcast biases) make every
   input DMA contiguous at 2KB/partition; all inputs are prefetched up
   front in dependency order behind a 28-matmul PE warm-up that bridges the
   ~6.5us engine preamble + first-DMA latency.
"""
import sys

if '/opt/trn_rl_repo' not in sys.path:
    sys.path.insert(0, '/opt/trn_rl_repo')

import numpy as np

import concourse.bass as bass
import concourse.mybir as mybir
import concourse.tile as tile
from concourse.vector_clock import ScopedClock
from concourse.masks import make_identity
from concourse.bass_utils import run_bass_kernel_spmd

F32 = mybir.dt.float32
F32R = mybir.dt.float32r
BF16 = mybir.dt.float16  # attention-path dtype (fp16: same PE rate, more mantissa)

D = 1024          # d_model
NH = 16           # heads
DK = 64           # head dim
BS = 128          # attention block size
N_CORES = 8
TOK = 2048        # tokens per core
ST = 512          # supertile tokens
NST = TOK // ST   # supertiles per core
SCALE = 1.0 / 8.0  # 1/sqrt(DK)

_MAX_DRAIN_WAITS = 1


class _SplitDrainTileContext(tile.TileContext):
    """The walrus in this container rejects >1 sync-wait on a NO_STRUCT
    instruction; Tile's exit drain waits on the whole global clock. Spread
    the waits across a chain of drains."""

    def _drain_and_barrier(self, tick_clock, wait_clock):
        nc = self.nc
        probe = nc.sync.drain()
        wait_clock.add_sem_waits(probe.ins, ScopedClock({None: tick_clock.global_clock}))
        si = probe.ins.sync_info
        waits = list(si.on_wait) if (si and si.on_wait) else []
        if len(waits) > _MAX_DRAIN_WAITS:
            probe.ins.sync_info = mybir.SyncInfo(
                on_wait=waits[:_MAX_DRAIN_WAITS],
                on_update=list(si.on_update) if si.on_update else [],
            )
            # Round-robin the remaining waits across all engines: each
            # engine's drains serialize, but five engines in parallel cut
            # the exit chain ~5x. The following barrier joins them.
            engs = [nc.vector, nc.scalar, nc.gpsimd, nc.tensor, nc.sync]
            for j, i in enumerate(
                    range(_MAX_DRAIN_WAITS, len(waits), _MAX_DRAIN_WAITS)):
                d = engs[j % len(engs)].drain()
                d.ins.sync_info = mybir.SyncInfo(
                    on_wait=waits[i:i + _MAX_DRAIN_WAITS], on_update=[]
                )
        nc.all_engine_barrier()
        assert self.sems is not None
        popped = nc._tile_sem_poison_stack.pop()
        assert popped is self._sem_poison
        nc.clear_and_free_semaphores(list(self.sems.allocated().values()))
        nc.all_engine_barrier()


def _split_excess_waits(nc, limit=1):
    """The nix walrus rejects instructions carrying more than `limit` sync
    waits. Hoist excess waits onto EventSemaphore instructions inserted just
    before, on the same (in-order) engine — semantics preserved."""
    n_split = 0
    for f in nc.m.functions:
        for bb in f.blocks:
            new = []
            changed = False
            for inst in bb.instructions:
                si = inst.sync_info
                waits = list(si.on_wait) if (si and si.on_wait) else []
                if len(waits) > limit:
                    excess = waits[:-limit]
                    for i in range(0, len(excess), limit):
                        ev = mybir.InstEventSemaphore(
                            name=f'I-splitw-{nc.next_id()}')
                        ev.engine = inst.engine
                        ev.sync_info = mybir.SyncInfo(
                            on_wait=excess[i:i + limit], on_update=[])
                        new.append(ev)
                        n_split += 1
                    inst.sync_info = mybir.SyncInfo(
                        on_wait=waits[-limit:],
                        on_update=list(si.on_update) if si.on_update else [])
                    changed = True
                new.append(inst)
            if changed:
                bb.instructions = new
    return n_split


def build_bass(split_waits=True):
    nc = bass.Bass('TRN2', target_bir_lowering=False, num_devices=N_CORES)

    xt_d = nc.dram_tensor('xt', [D, TOK], BF16, kind='ExternalInput')
    # weights host-pre-arranged as [m, p, c, n] so each per-m-chunk DMA is a
    # fully contiguous [128, 1024] fp16 copy (2KB/partition descriptors).
    wq_d = nc.dram_tensor('wq', [8, 128, D], BF16, kind='ExternalInput')
    wk_d = nc.dram_tensor('wk', [8, 128, D], BF16, kind='ExternalInput')
    wv_d = nc.dram_tensor('wv', [8, 128, D], BF16, kind='ExternalInput')
    wo_d = nc.dram_tensor('wo', [8, 128, D], BF16, kind='ExternalInput')
    bq_d = nc.dram_tensor('bq', [128, 8], F32, kind='ExternalInput')
    bk_d = nc.dram_tensor('bk', [128, 8], F32, kind='ExternalInput')
    bv_d = nc.dram_tensor('bv', [128, D], F32, kind='ExternalInput')
    bo_d = nc.dram_tensor('bo', [1, D], BF16, kind='ExternalInput')
    bobc_d = nc.dram_tensor('bobc', [128, D], F32, kind='ExternalInput')
    ones_d = nc.dram_tensor('ones', [1, 128], BF16, kind='ExternalInput')
    out_d = nc.dram_tensor('out', [TOK, D], F32, kind='ExternalOutput')

    with _SplitDrainTileContext(nc) as tc:
        _build_body(nc, tc, xt_d, wq_d, wk_d, wv_d, wo_d,
                    bq_d, bk_d, bv_d, bo_d, bobc_d, ones_d, out_d)
    if split_waits:
        # CoreSim chokes on the inserted EventSemaphores; only split for HW.
        _split_excess_waits(nc, limit=1)
    return nc


def _build_body(nc, tc, xt_d, wq_d, wk_d, wv_d, wo_d, bq_d, bk_d, bv_d, bo_d, bobc_d, ones_d, out_d):
    from contextlib import ExitStack
    with ExitStack() as ctx:
        _build_pools_and_body(nc, tc, ctx, xt_d, wq_d, wk_d, wv_d, wo_d,
                              bq_d, bk_d, bv_d, bo_d, bobc_d, ones_d, out_d)


def _build_pools_and_body(nc, tc, ctx, xt_d, wq_d, wk_d, wv_d, wo_d,
                          bq_d, bk_d, bv_d, bo_d, bobc_d, ones_d, out_d):
    AF = mybir.ActivationFunctionType
    OP = mybir.AluOpType
    AX = mybir.AxisListType

    wpool = ctx.enter_context(tc.tile_pool(name='w', bufs=1))
    cpool = ctx.enter_context(tc.tile_pool(name='c', bufs=1))
    xpool = ctx.enter_context(tc.tile_pool(name='x', bufs=1))
    qkv = ctx.enter_context(tc.tile_pool(name='qkv', bufs=1))
    apool = ctx.enter_context(tc.tile_pool(name='a', bufs=2))
    opool = ctx.enter_context(tc.tile_pool(name='o', bufs=2))
    otpool = ctx.enter_context(tc.tile_pool(name='ot', bufs=2))

    # One shared 4-deep ring of 2KB PSUM slots serves both the projection
    # matmuls and the attention scores (they occupy disjoint phases); pat
    # holds transposed-A (fp16), pav the two A@V accumulators.
    pp = ctx.enter_context(tc.tile_pool(name='pp', bufs=4, space='PSUM'))
    pat = ctx.enter_context(tc.tile_pool(name='pat', bufs=2, space='PSUM'))
    pav = ctx.enter_context(tc.tile_pool(name='pav', bufs=1, space='PSUM'))

    # ---- constants / weights ----
    # DMA queue order IS the arrival order: xt0 and wq first (they gate the
    # first real matmuls), then the small constants, then wk/wv/wo, then the
    # remaining supertiles' activations. All transfers are contiguous
    # (host-side pre-layout), so the queue runs at full HBM rate.
    xt_tiles = [None] * NST
    xt_tiles[0] = xpool.tile([128, 8, ST], BF16, name='xt', tag='xt0')
    nc.sync.dma_start(
        out=xt_tiles[0],
        in_=xt_d.ap()[:, 0:ST].rearrange('(c p) t -> p c t', p=128),
    )

    # w_sb layout [p, m, c, n]: the per-m DMA writes one contiguous
    # 2KB/partition block; the (c, m) matmul lhsT slice is [p, 128] strided.
    w_sb = {}
    for nm in ('q', 'k', 'v', 'o'):
        w_sb[nm] = wpool.tile([128, 8, 8, 128], BF16, name=f'w{nm}',
                              tag=f'w{nm}')

    def _load_w(nm, wd):
        for m in range(8):
            nc.sync.dma_start(
                out=w_sb[nm][:, m, :, :],
                in_=wd.ap()[m].rearrange('p (c n) -> p c n', c=8))

    _load_w('q', wq_d)

    bq_sb = cpool.tile([128, 8], F32, name='bq', tag='bq')
    nc.sync.dma_start(out=bq_sb, in_=bq_d.ap())
    bk_sb = cpool.tile([128, 8], F32, name='bk', tag='bk')
    nc.sync.dma_start(out=bk_sb, in_=bk_d.ap())
    bv_bc = cpool.tile([128, D], F32, name='bvbc', tag='bvbc')
    nc.sync.dma_start(out=bv_bc, in_=bv_d.ap())
    bo_sb = cpool.tile([1, D], BF16, name='bo', tag='bo')
    nc.sync.dma_start(out=bo_sb, in_=bo_d.ap())
    bo_bc = cpool.tile([128, D], F32, name='bobc', tag='bobc')
    nc.sync.dma_start(out=bo_bc, in_=bobc_d.ap())
    ones_sb = cpool.tile([1, 128], BF16, name='ones', tag='ones')
    nc.sync.dma_start(out=ones_sb, in_=ones_d.ap())

    ident = cpool.tile([128, 128], BF16, name='ident', tag='ident')
    make_identity(nc, ident)

    # PE warm-up: HAM un-throttles only after ~3.4us of sustained activity,
    # and the DMA pipe needs ~13us before xt0+wq have landed. Dummy matmuls
    # rotate through the shared PSUM ring so they issue back-to-back.
    warm_sb = cpool.tile([128, 512], BF16, name='warm', tag='warm')
    nc.vector.memset(warm_sb, 0.5)
    for _ in range(36):
        ps_warm = pp.tile([128, ST], F32, name='ps', tag='ps')
        nc.tensor.matmul(ps_warm, lhsT=warm_sb[:, 0:128], rhs=warm_sb,
                         start=True, stop=True)

    _load_w('k', wk_d)
    _load_w('v', wv_d)
    _load_w('o', wo_d)

    for s in range(1, NST):
        xt_tiles[s] = xpool.tile([128, 8, ST], BF16, name='xt', tag=f'xt{s}')
        nc.sync.dma_start(
            out=xt_tiles[s],
            in_=xt_d.ap()[:, s * ST:(s + 1) * ST].rearrange(
                '(c p) t -> p c t', p=128),
        )

    def r_(ap):
        return ap

    def r_(ap):
        return ap

    # ---- cross-supertile software pipeline ----
    # Supertile s's attention is interleaved instruction-by-instruction with
    # supertile s+1's projections: the projection matmuls are pure-PE filler
    # that hides the attention's cross-engine softmax chains AND keeps the
    # PE dense enough that HAM never re-throttles. The per-engine order is
    # static at runtime, so this must be done at emission time.
    proj_state = {}

    def emit_proj_unit(s, idx):
        """idx 0..7: Q chunk m; 8..15: K chunk m; 16..23: V chunk."""
        st = proj_state[s]
        xt_sb, qt_sb, kt_sb, v_sb = st['xt'], st['qt'], st['kt'], st['v']
        if idx < 8:
            m = idx
            ps = pp.tile([128, ST], F32, name='ps', tag='ps')
            for c in range(8):
                nc.tensor.matmul(ps, lhsT=r_(w_sb['q'][:, m, c, :]),
                                 rhs=r_(xt_sb[:, c, :]),
                                 start=(c == 0), stop=(c == 7))
            nc.vector.tensor_scalar(out=qt_sb[:, m, :], in0=ps,
                                    scalar1=bq_sb[:, m:m + 1], scalar2=SCALE,
                                    op0=OP.add, op1=OP.mult)
        elif idx < 16:
            m = idx - 8
            ps = pp.tile([128, ST], F32, name='ps', tag='ps')
            for c in range(8):
                nc.tensor.matmul(ps, lhsT=r_(w_sb['k'][:, m, c, :]),
                                 rhs=r_(xt_sb[:, c, :]),
                                 start=(c == 0), stop=(c == 7))
            nc.vector.tensor_scalar(out=kt_sb[:, m, :], in0=ps,
                                    scalar1=bk_sb[:, m:m + 1], scalar2=None,
                                    op0=OP.add)
        else:
            tch, nh2 = divmod(idx - 16, 2)
            ps = pp.tile([128, ST], F32, name='ps', tag='ps')
            for c in range(8):
                nc.tensor.matmul(
                    ps, lhsT=r_(xt_sb[:, c, tch * 128:(tch + 1) * 128]),
                    rhs=r_(w_sb['v'][:, nh2 * 4:(nh2 + 1) * 4, c, :]),
                    start=(c == 0), stop=(c == 7))
            nc.vector.tensor_tensor(
                out=v_sb[:, tch, nh2 * 512:(nh2 + 1) * 512], in0=ps,
                in1=bv_bc[:, nh2 * 512:(nh2 + 1) * 512], op=OP.add)

    def open_proj(s):
        proj_state[s] = {
            'xt': xt_tiles[s],
            'qt': qkv.tile([128, 8, ST], BF16, name='qt', tag='qt', bufs=2),
            'kt': qkv.tile([128, 8, ST], BF16, name='kt', tag='kt', bufs=2),
            'v': qkv.tile([128, 4, D], BF16, name='v', tag='v', bufs=2),
        }

    def _groups():
        for g in range(4):
            parity = g % 2
            base = (g // 2) * 8
            yield ([base + parity + 2 * i for i in range(4)], parity * 64)

    attn_state = {}

    def S_phase(s, b4):
        # Heads grouped by parity: every scores matmul in a group reads
        # Q^T/K^T at the SAME partition offset; mixing offsets across
        # matmuls that write one PSUM bank wedges the device.
        st = proj_state[s]
        qt_sb, kt_sb = st['qt'], st['kt']
        t0 = b4 * 128
        e_tiles = []
        for heads, off in _groups():
            ps_sc = pp.tile([128, 4, 128], F32, name='ps_sc', tag='ps')
            for i, hh in enumerate(heads):
                m = hh // 2
                nc.tensor.matmul(
                    ps_sc[:, i, :],
                    lhsT=qt_sb[off:off + 64, m, t0:t0 + 128],
                    rhs=kt_sb[off:off + 64, m, t0:t0 + 128],
                    start=True, stop=True)
            e_sb = apool.tile([128, 4, 128], BF16, name='e', tag='e', bufs=8)
            nc.scalar.activation(e_sb, ps_sc, AF.Exp)
            stat = apool.tile([128, 8], F32, name='stat', tag='stat', bufs=8)
            nc.vector.reduce_sum(out=stat[:, 0:4], in_=e_sb, axis=AX.X)
            nc.vector.reciprocal(stat[:, 4:8], stat[:, 0:4])
            nc.gpsimd.tensor_tensor(
                out=e_sb, in0=e_sb,
                in1=stat[:, 4:8].to_broadcast((128, 4, 128)),
                op=OP.mult)
            e_tiles.append(e_sb)
        attn_state[(s, b4)] = e_tiles

    def T_phase(s, b4):
        e_tiles = attn_state.pop((s, b4))
        v_sb = proj_state[s]['v']
        ps_av0 = pav.tile([128, 4, 128], F32, name='ps_av0')
        ps_av1 = pav.tile([128, 4, 128], F32, name='ps_av1')
        ot_sb = otpool.tile([128, 8, 128], BF16, name='ot')
        for g, (heads, off) in enumerate(_groups()):
            e_sb = e_tiles[g]
            ps_at = pat.tile([128, 4, 128], BF16, name='ps_at')
            for i in range(4):
                nc.tensor.transpose(ps_at[:, i, :], e_sb[:, i, :], ident)
            at_sb = apool.tile([128, 4, 128], BF16, name='at', tag='at',
                               bufs=4)
            nc.scalar.copy(at_sb, ps_at)
            for i, hh in enumerate(heads):
                g2 = hh // 2
                ps_av = ps_av0 if g2 < 4 else ps_av1
                nc.tensor.matmul(
                    ps_av[off:off + 64, g2 % 4, :],
                    lhsT=v_sb[:, b4, hh * 64:(hh + 1) * 64],
                    rhs=at_sb[:, i, :],
                    start=True, stop=True)
            if g == 1:
                nc.vector.tensor_copy(ot_sb[:, 0:4, :], ps_av0)
            elif g == 3:
                nc.vector.tensor_copy(ot_sb[:, 4:8, :], ps_av1)
        return ot_sb

    def O_phase(s, b4, ot_sb, bias_mm=False):
        # bias via the DVE eviction add normally (saves a K=1 PE matmul per
        # half); via the ones-matmul for the unpaired tail blocks where the
        # DVE is the jammed engine and the PE has slack.
        t0 = b4 * 128
        for nh2 in range(2):
            ps = pp.tile([128, ST], F32, name='ps', tag='ps')
            for c in range(8):
                nc.tensor.matmul(
                    ps, lhsT=r_(ot_sb[:, c, :]),
                    rhs=r_(w_sb['o'][:, nh2 * 4:(nh2 + 1) * 4, c, :]),
                    start=(c == 0), stop=(not bias_mm and c == 7))
            out_sb = opool.tile([128, 512], F32, name='outsb')
            if bias_mm:
                nc.tensor.matmul(ps, lhsT=r_(ones_sb),
                                 rhs=r_(bo_sb[:, nh2 * 512:(nh2 + 1) * 512]),
                                 start=False, stop=True)
                nc.vector.tensor_copy(out_sb, ps)
            else:
                nc.vector.tensor_tensor(
                    out=out_sb, in0=ps,
                    in1=bo_bc[:, nh2 * 512:(nh2 + 1) * 512], op=OP.add)
            nc.sync.dma_start(
                out=out_d.ap()[s * ST + t0: s * ST + t0 + 128,
                               nh2 * 512:(nh2 + 1) * 512],
                in_=out_sb)

    def emit_attn_unit(s, u):
        """12 units per supertile: interleavable S/T/O phases of 4 blocks,
        ordered so every T sits a full S-phase behind its scores."""
        seq = [('S', 0), ('S', 1), ('T', 0), ('O', 0), ('S', 2), ('T', 1),
               ('O', 1), ('S', 3), ('T', 2), ('O', 2), ('T', 3), ('O', 3)]
        kind, b = seq[u]
        if kind == 'S':
            S_phase(s, b)
        elif kind == 'T':
            attn_state[('ot', s, b)] = T_phase(s, b)
        else:
            O_phase(s, b, attn_state.pop(('ot', s, b)))

    # ST0 projections run alone (behind the warmup); thereafter supertile
    # s's projection chunks interleave with supertile s-1's 12 attention
    # units. For the last supertile only Q/K interleave there; its V chunks
    # are saved to cover the last supertile's own early attention blocks, so
    # the unpaired engine-jammed tail shrinks from 4 blocks to ~2.
    open_proj(0)
    for idx in range(24):
        emit_proj_unit(0, idx)
    LAST = NST - 1
    for s in range(1, NST):
        open_proj(s)
        n_front = 16 if s == LAST else 24
        counts = [2] * 12
        if s == LAST:
            counts = [2, 2, 1, 1, 2, 1, 1, 2, 1, 1, 1, 1]
        pi = 0
        for u in range(12):
            emit_attn_unit(s - 1, u)
            for _ in range(counts[u]):
                if pi < n_front:
                    emit_proj_unit(s, pi); pi += 1
        while pi < n_front:
            emit_proj_unit(s, pi); pi += 1
    # Final supertile: V-projection chunks (16..23) interleaved with its
    # attention. T(3,b) needs v[:, b, :] = V units 16+2b, 17+2b.
    s = LAST
    S_phase(s, 0)
    emit_proj_unit(s, 16); emit_proj_unit(s, 17)
    S_phase(s, 1)
    emit_proj_unit(s, 18); emit_proj_unit(s, 19)
    ot0 = T_phase(s, 0)
    emit_proj_unit(s, 20)
    O_phase(s, 0, ot0)
    emit_proj_unit(s, 21)
    S_phase(s, 2)
    emit_proj_unit(s, 22)
    ot1 = T_phase(s, 1)
    emit_proj_unit(s, 23)
    O_phase(s, 1, ot1)
    S_phase(s, 3)
    ot2 = T_phase(s, 2)
    O_phase(s, 2, ot2, bias_mm=True)
    ot3 = T_phase(s, 3)
    O_phase(s, 3, ot3, bias_mm=True)


_NC_CACHE = []


def _get_nc():
    if not _NC_CACHE:
        _NC_CACHE.append(build_bass())
    return _NC_CACHE[0]


def shard_inputs(x, Wq, bq, Wk, bk, Wv, bv, Wo, bo):
    x = np.asarray(x, dtype=np.float32)
    B, S, _ = x.shape
    xf = np.ascontiguousarray(x.reshape(B * S, D))
    assert B * S == N_CORES * TOK

    def warr(W):
        # [m, p, c, n] = W[c*128+p, m*128+n]: every per-m DMA is contiguous.
        W = np.asarray(W, dtype=np.float16)
        return np.ascontiguousarray(
            W.reshape(8, 128, 8, 128).transpose(2, 1, 0, 3).reshape(8, 128, D))

    shared = {
        'wq': warr(Wq),
        'wk': warr(Wk),
        'wv': warr(Wv),
        'wo': warr(Wo),
        'bq': np.ascontiguousarray(np.asarray(bq, dtype=np.float32).reshape(8, 128).T),
        'bk': np.ascontiguousarray(np.asarray(bk, dtype=np.float32).reshape(8, 128).T),
        'bv': np.ascontiguousarray(
            np.broadcast_to(np.asarray(bv, dtype=np.float32).reshape(1, D),
                            (128, D))),
        'bo': np.ascontiguousarray(np.asarray(bo, dtype=np.float16).reshape(1, D)),
        'bobc': np.ascontiguousarray(
            np.broadcast_to(np.asarray(bo, dtype=np.float32).reshape(1, D),
                            (128, D))),
        'ones': np.ones((1, 128), dtype=np.float16),
    }
    in_maps = []
    for c in range(N_CORES):
        xt = np.ascontiguousarray(xf[c * TOK:(c + 1) * TOK, :].T.astype(np.float16))
        in_maps.append({'xt': xt, **shared})
    return (B, S), in_maps


def run(inputs, **spmd_kwargs):
    (B, S), in_maps = shard_inputs(**inputs)
    nc = _get_nc()
    res = run_bass_kernel_spmd(nc, in_maps, list(range(N_CORES)), **spmd_kwargs)
    out = np.concatenate([res.results[c]['out'] for c in range(N_CORES)], axis=0)
    return out.reshape(B, S, D), res


def kernel(x, Wq, bq, Wk, bk, Wv, bv, Wo, bo):
    out, _ = run(dict(x=x, Wq=Wq, bq=bq, Wk=Wk, bk=bk,
                      Wv=Wv, bv=bv, Wo=Wo, bo=bo))
    return out



# revision 30
# speedup vs baseline: 1.1877x; 1.0161x over previous
"""Block-sparse attention (block-local) Bass kernel for 8 Trainium2 NeuronCores.

Problem: x[4, 4096, 1024] -> 4 linear projections (Q/K/V/O) + block-local
attention (block size 128, 16 heads, d_k 64), all f32.

Sharding: pure data parallel over tokens. Attention is block-local with
block size 128, so the flattened token axis [16384] splits across 8 cores
into 2048-token shards (16 blocks each) with zero cross-core communication.

Per-core kernel layout strategy:
 - x is passed host-transposed as xT [1024, 2048] so activations live in
   SBUF with d_model on partitions; Q/K projections then need no on-chip
   transposes (matmul(lhsT=W_chunk, rhs=xT_chunk)). All matmul data is fp16.
 - Q^T/K^T produced in [d_model, token] layout (what scores matmuls need),
   V in natural [token, d_model] layout (what the A@V matmul needs).
 - Per 128-token block: scores -> exp (ACT) -> row-sum (DVE) -> normalize
   (GpSimd) in [q, k] layout, then PE-transpose of A to feed A@V, whose
   [d, q] output is exactly the lhsT the final Wo projection needs.
 - The per-engine instruction order is static at runtime, so the kernel is
   software-pipelined at emission time: supertile s's attention interleaves
   with supertile s+1's projection matmuls (pure-PE filler that hides the
   cross-engine softmax chains and keeps HAM at 8/8); the last supertile's
   attention interleaves with its own V-projection chunks.
 - Host pre-layouts (weights as [m, p, c, n], broadcast biases) make every
   input DMA contiguous at 2KB/partition; all inputs are prefetched up
   front in dependency order behind a 36-matmul PE warm-up that bridges the
   ~6.5us engine preamble + first-DMA latency.
"""
import sys

if '/opt/trn_rl_repo' not in sys.path:
    sys.path.insert(0, '/opt/trn_rl_repo')

import numpy as np

import concourse.bass as bass
import concourse.mybir as mybir
import concourse.tile as tile
from concourse.vector_clock import ScopedClock
from concourse.masks import make_identity
from concourse.bass_utils import run_bass_kernel_spmd

F32 = mybir.dt.float32
F32R = mybir.dt.float32r
BF16 = mybir.dt.float16  # attention-path dtype (fp16: same PE rate, more mantissa)

D = 1024          # d_model
NH = 16           # heads
DK = 64           # head dim
BS = 128          # attention block size
N_CORES = 8
TOK = 2048        # tokens per core
ST = 512          # supertile tokens
NST = TOK // ST   # supertiles per core
SCALE = 1.0 / 8.0  # 1/sqrt(DK)

_MAX_DRAIN_WAITS = 1


class _SplitDrainTileContext(tile.TileContext):
    """The walrus in this container rejects >1 sync-wait on a NO_STRUCT
    instruction; Tile's exit drain waits on the whole global clock. Spread
    the waits across a chain of drains."""

    def _drain_and_barrier(self, tick_clock, wait_clock):
        nc = self.nc
        probe = nc.sync.drain()
        wait_clock.add_sem_waits(probe.ins, ScopedClock({None: tick_clock.global_clock}))
        si = probe.ins.sync_info
        waits = list(si.on_wait) if (si and si.on_wait) else []
        if len(waits) > _MAX_DRAIN_WAITS:
            probe.ins.sync_info = mybir.SyncInfo(
                on_wait=waits[:_MAX_DRAIN_WAITS],
                on_update=list(si.on_update) if si.on_update else [],
            )
            # Round-robin the remaining waits across all engines: each
            # engine's drains serialize, but five engines in parallel cut
            # the exit chain ~5x. The following barrier joins them.
            engs = [nc.vector, nc.scalar, nc.gpsimd, nc.tensor, nc.sync]
            for j, i in enumerate(
                    range(_MAX_DRAIN_WAITS, len(waits), _MAX_DRAIN_WAITS)):
                d = engs[j % len(engs)].drain()
                d.ins.sync_info = mybir.SyncInfo(
                    on_wait=waits[i:i + _MAX_DRAIN_WAITS], on_update=[]
                )
        nc.all_engine_barrier()
        assert self.sems is not None
        popped = nc._tile_sem_poison_stack.pop()
        assert popped is self._sem_poison
        nc.clear_and_free_semaphores(list(self.sems.allocated().values()))
        nc.all_engine_barrier()


def _split_excess_waits(nc, limit=1):
    """The nix walrus rejects instructions carrying more than `limit` sync
    waits. Hoist excess waits onto EventSemaphore instructions inserted just
    before, on the same (in-order) engine — semantics preserved."""
    n_split = 0
    for f in nc.m.functions:
        for bb in f.blocks:
            new = []
            changed = False
            for inst in bb.instructions:
                si = inst.sync_info
                waits = list(si.on_wait) if (si and si.on_wait) else []
                if len(waits) > limit:
                    excess = waits[:-limit]
                    for i in range(0, len(excess), limit):
                        ev = mybir.InstEventSemaphore(
                            name=f'I-splitw-{nc.next_id()}')
                        ev.engine = inst.engine
                        ev.sync_info = mybir.SyncInfo(
                            on_wait=excess[i:i + limit], on_update=[])
                        new.append(ev)
                        n_split += 1
                    inst.sync_info = mybir.SyncInfo(
                        on_wait=waits[-limit:],
                        on_update=list(si.on_update) if si.on_update else [])
                    changed = True
                new.append(inst)
            if changed:
                bb.instructions = new
    return n_split


def build_bass(split_waits=True):
    nc = bass.Bass('TRN2', target_bir_lowering=False, num_devices=N_CORES)

    xt_d = nc.dram_tensor('xt', [D, TOK], BF16, kind='ExternalInput')
    # weights host-pre-arranged as [m, p, c, n] so each per-m-chunk DMA is a
    # fully contiguous [128, 1024] fp16 copy (2KB/partition descriptors).
    wq_d = nc.dram_tensor('wq', [8, 128, D], BF16, kind='ExternalInput')
    wk_d = nc.dram_tensor('wk', [8, 128, D], BF16, kind='ExternalInput')
    wv_d = nc.dram_tensor('wv', [8, 128, D], BF16, kind='ExternalInput')
    wo_d = nc.dram_tensor('wo', [8, 128, D], BF16, kind='ExternalInput')
    bq_d = nc.dram_tensor('bq', [128, 8], F32, kind='ExternalInput')
    bk_d = nc.dram_tensor('bk', [128, 8], F32, kind='ExternalInput')
    bv_d = nc.dram_tensor('bv', [128, D], F32, kind='ExternalInput')
    bo_d = nc.dram_tensor('bo', [1, D], BF16, kind='ExternalInput')
    bobc_d = nc.dram_tensor('bobc', [128, D], F32, kind='ExternalInput')
    ones_d = nc.dram_tensor('ones', [1, 128], BF16, kind='ExternalInput')
    out_d = nc.dram_tensor('out', [TOK, D], F32, kind='ExternalOutput')

    with _SplitDrainTileContext(nc) as tc:
        _build_body(nc, tc, xt_d, wq_d, wk_d, wv_d, wo_d,
                    bq_d, bk_d, bv_d, bo_d, bobc_d, ones_d, out_d)
    if split_waits:
        # CoreSim chokes on the inserted EventSemaphores; only split for HW.
        _split_excess_waits(nc, limit=1)
    return nc


def _build_body(nc, tc, xt_d, wq_d, wk_d, wv_d, wo_d, bq_d, bk_d, bv_d, bo_d, bobc_d, ones_d, out_d):
    from contextlib import ExitStack
    with ExitStack() as ctx:
        _build_pools_and_body(nc, tc, ctx, xt_d, wq_d, wk_d, wv_d, wo_d,
                              bq_d, bk_d, bv_d, bo_d, bobc_d, ones_d, out_d)


def _build_pools_and_body(nc, tc, ctx, xt_d, wq_d, wk_d, wv_d, wo_d,
                          bq_d, bk_d, bv_d, bo_d, bobc_d, ones_d, out_d):
    AF = mybir.ActivationFunctionType
    OP = mybir.AluOpType
    AX = mybir.AxisListType

    wpool = ctx.enter_context(tc.tile_pool(name='w', bufs=1))
    cpool = ctx.enter_context(tc.tile_pool(name='c', bufs=1))
    xpool = ctx.enter_context(tc.tile_pool(name='x', bufs=1))
    qkv = ctx.enter_context(tc.tile_pool(name='qkv', bufs=1))
    apool = ctx.enter_context(tc.tile_pool(name='a', bufs=2))
    opool = ctx.enter_context(tc.tile_pool(name='o', bufs=2))
    otpool = ctx.enter_context(tc.tile_pool(name='ot', bufs=2))

    # One shared 4-deep ring of 2KB PSUM slots serves both the projection
    # matmuls and the attention scores (they occupy disjoint phases); pat
    # holds transposed-A (fp16), pav the two A@V accumulators.
    pp = ctx.enter_context(tc.tile_pool(name='pp', bufs=4, space='PSUM'))
    pat = ctx.enter_context(tc.tile_pool(name='pat', bufs=2, space='PSUM'))
    pav = ctx.enter_context(tc.tile_pool(name='pav', bufs=1, space='PSUM'))

    # ---- constants / weights ----
    # DMA queue order IS the arrival order: xt0 and wq first (they gate the
    # first real matmuls), then the small constants, then wk/wv/wo, then the
    # remaining supertiles' activations. All transfers are contiguous
    # (host-side pre-layout), so the queue runs at full HBM rate.
    xt_tiles = [None] * NST
    xt_tiles[0] = xpool.tile([128, 8, ST], BF16, name='xt', tag='xt0')
    nc.sync.dma_start(
        out=xt_tiles[0],
        in_=xt_d.ap()[:, 0:ST].rearrange('(c p) t -> p c t', p=128),
    )

    # w_sb layout [p, m, c, n]: the per-m DMA writes one contiguous
    # 2KB/partition block; the (c, m) matmul lhsT slice is [p, 128] strided.
    w_sb = {}
    for nm in ('q', 'k', 'v', 'o'):
        w_sb[nm] = wpool.tile([128, 8, 8, 128], BF16, name=f'w{nm}',
                              tag=f'w{nm}')

    def _load_w(nm, wd):
        for m in range(8):
            nc.sync.dma_start(
                out=w_sb[nm][:, m, :, :],
                in_=wd.ap()[m].rearrange('p (c n) -> p c n', c=8))

    _load_w('q', wq_d)

    bq_sb = cpool.tile([128, 8], F32, name='bq', tag='bq')
    nc.sync.dma_start(out=bq_sb, in_=bq_d.ap())
    bk_sb = cpool.tile([128, 8], F32, name='bk', tag='bk')
    nc.sync.dma_start(out=bk_sb, in_=bk_d.ap())
    bv_bc = cpool.tile([128, D], F32, name='bvbc', tag='bvbc')
    nc.sync.dma_start(out=bv_bc, in_=bv_d.ap())
    bo_sb = cpool.tile([1, D], BF16, name='bo', tag='bo')
    nc.sync.dma_start(out=bo_sb, in_=bo_d.ap())
    bo_bc = cpool.tile([128, D], F32, name='bobc', tag='bobc')
    nc.sync.dma_start(out=bo_bc, in_=bobc_d.ap())
    ones_sb = cpool.tile([1, 128], BF16, name='ones', tag='ones')
    nc.sync.dma_start(out=ones_sb, in_=ones_d.ap())

    ident = cpool.tile([128, 128], BF16, name='ident', tag='ident')
    make_identity(nc, ident)

    # PE warm-up: HAM un-throttles only after ~3.4us of sustained activity,
    # and the DMA pipe needs ~13us before xt0+wq have landed. Dummy matmuls
    # rotate through the shared PSUM ring so they issue back-to-back.
    warm_sb = cpool.tile([128, 512], BF16, name='warm', tag='warm')
    nc.vector.memset(warm_sb, 0.5)
    for _ in range(36):
        ps_warm = pp.tile([128, ST], F32, name='ps', tag='ps')
        nc.tensor.matmul(ps_warm, lhsT=warm_sb[:, 0:128], rhs=warm_sb,
                         start=True, stop=True)

    _load_w('k', wk_d)
    _load_w('v', wv_d)
    _load_w('o', wo_d)

    for s in range(1, NST):
        xt_tiles[s] = xpool.tile([128, 8, ST], BF16, name='xt', tag=f'xt{s}')
        nc.sync.dma_start(
            out=xt_tiles[s],
            in_=xt_d.ap()[:, s * ST:(s + 1) * ST].rearrange(
                '(c p) t -> p c t', p=128),
        )

    def r_(ap):
        return ap

    def r_(ap):
        return ap

    # ---- cross-supertile software pipeline ----
    # Supertile s's attention is interleaved instruction-by-instruction with
    # supertile s+1's projections: the projection matmuls are pure-PE filler
    # that hides the attention's cross-engine softmax chains AND keeps the
    # PE dense enough that HAM never re-throttles. The per-engine order is
    # static at runtime, so this must be done at emission time.
    proj_state = {}

    def emit_proj_unit(s, idx):
        """idx 0..7: Q chunk m; 8..15: K chunk m; 16..23: V chunk."""
        st = proj_state[s]
        xt_sb, qt_sb, kt_sb, v_sb = st['xt'], st['qt'], st['kt'], st['v']
        if idx < 8:
            m = idx
            ps = pp.tile([128, ST], F32, name='ps', tag='ps')
            for c in range(8):
                nc.tensor.matmul(ps, lhsT=r_(w_sb['q'][:, m, c, :]),
                                 rhs=r_(xt_sb[:, c, :]),
                                 start=(c == 0), stop=(c == 7))
            nc.vector.tensor_scalar(out=qt_sb[:, m, :], in0=ps,
                                    scalar1=bq_sb[:, m:m + 1], scalar2=SCALE,
                                    op0=OP.add, op1=OP.mult)
        elif idx < 16:
            m = idx - 8
            ps = pp.tile([128, ST], F32, name='ps', tag='ps')
            for c in range(8):
                nc.tensor.matmul(ps, lhsT=r_(w_sb['k'][:, m, c, :]),
                                 rhs=r_(xt_sb[:, c, :]),
                                 start=(c == 0), stop=(c == 7))
            nc.vector.tensor_scalar(out=kt_sb[:, m, :], in0=ps,
                                    scalar1=bk_sb[:, m:m + 1], scalar2=None,
                                    op0=OP.add)
        else:
            tch, nh2 = divmod(idx - 16, 2)
            ps = pp.tile([128, ST], F32, name='ps', tag='ps')
            for c in range(8):
                nc.tensor.matmul(
                    ps, lhsT=r_(xt_sb[:, c, tch * 128:(tch + 1) * 128]),
                    rhs=r_(w_sb['v'][:, nh2 * 4:(nh2 + 1) * 4, c, :]),
                    start=(c == 0), stop=(c == 7))
            nc.vector.tensor_tensor(
                out=v_sb[:, tch, nh2 * 512:(nh2 + 1) * 512], in0=ps,
                in1=bv_bc[:, nh2 * 512:(nh2 + 1) * 512], op=OP.add)

    def open_proj(s):
        proj_state[s] = {
            'xt': xt_tiles[s],
            'qt': qkv.tile([128, 8, ST], BF16, name='qt', tag='qt', bufs=2),
            'kt': qkv.tile([128, 8, ST], BF16, name='kt', tag='kt', bufs=2),
            'v': qkv.tile([128, 4, D], BF16, name='v', tag='v', bufs=2),
        }

    def _groups():
        for g in range(4):
            parity = g % 2
            base = (g // 2) * 8
            yield ([base + parity + 2 * i for i in range(4)], parity * 64)

    attn_state = {}

    def S_phase(s, b4):
        # Heads grouped by parity: every scores matmul in a group reads
        # Q^T/K^T at the SAME partition offset; mixing offsets across
        # matmuls that write one PSUM bank wedges the device.
        st = proj_state[s]
        qt_sb, kt_sb = st['qt'], st['kt']
        t0 = b4 * 128
        e_tiles = []
        for heads, off in _groups():
            ps_sc = pp.tile([128, 4, 128], F32, name='ps_sc', tag='ps')
            for i, hh in enumerate(heads):
                m = hh // 2
                nc.tensor.matmul(
                    ps_sc[:, i, :],
                    lhsT=qt_sb[off:off + 64, m, t0:t0 + 128],
                    rhs=kt_sb[off:off + 64, m, t0:t0 + 128],
                    start=True, stop=True)
            e_sb = apool.tile([128, 4, 128], BF16, name='e', tag='e', bufs=8)
            nc.scalar.activation(e_sb, ps_sc, AF.Exp)
            stat = apool.tile([128, 8], F32, name='stat', tag='stat', bufs=8)
            nc.vector.reduce_sum(out=stat[:, 0:4], in_=e_sb, axis=AX.X)
            nc.vector.reciprocal(stat[:, 4:8], stat[:, 0:4])
            nc.gpsimd.tensor_tensor(
                out=e_sb, in0=e_sb,
                in1=stat[:, 4:8].to_broadcast((128, 4, 128)),
                op=OP.mult)
            e_tiles.append(e_sb)
        attn_state[(s, b4)] = e_tiles

    def T_phase(s, b4):
        e_tiles = attn_state.pop((s, b4))
        v_sb = proj_state[s]['v']
        ps_av0 = pav.tile([128, 4, 128], F32, name='ps_av0')
        ps_av1 = pav.tile([128, 4, 128], F32, name='ps_av1')
        ot_sb = otpool.tile([128, 8, 128], BF16, name='ot', bufs=4)
        for g, (heads, off) in enumerate(_groups()):
            e_sb = e_tiles[g]
            ps_at = pat.tile([128, 4, 128], BF16, name='ps_at')
            for i in range(4):
                nc.tensor.transpose(ps_at[:, i, :], e_sb[:, i, :], ident)
            at_sb = apool.tile([128, 4, 128], BF16, name='at', tag='at',
                               bufs=4)
            nc.scalar.copy(at_sb, ps_at)
            for i, hh in enumerate(heads):
                g2 = hh // 2
                ps_av = ps_av0 if g2 < 4 else ps_av1
                nc.tensor.matmul(
                    ps_av[off:off + 64, g2 % 4, :],
                    lhsT=v_sb[:, b4, hh * 64:(hh + 1) * 64],
                    rhs=at_sb[:, i, :],
                    start=True, stop=True)
            if g == 1:
                nc.vector.tensor_copy(ot_sb[:, 0:4, :], ps_av0)
            elif g == 3:
                nc.vector.tensor_copy(ot_sb[:, 4:8, :], ps_av1)
        return ot_sb

    def O_phase(s, b4, ot_sb, bias_mm=False):
        # bias via the DVE eviction add normally (saves a K=1 PE matmul per
        # half); via the ones-matmul for the unpaired tail blocks where the
        # DVE is the jammed engine and the PE has slack.
        t0 = b4 * 128
        for nh2 in range(2):
            ps = pp.tile([128, ST], F32, name='ps', tag='ps')
            for c in range(8):
                nc.tensor.matmul(
                    ps, lhsT=r_(ot_sb[:, c, :]),
                    rhs=r_(w_sb['o'][:, nh2 * 4:(nh2 + 1) * 4, c, :]),
                    start=(c == 0), stop=(not bias_mm and c == 7))
            out_sb = opool.tile([128, 512], F32, name='outsb')
            if bias_mm:
                nc.tensor.matmul(ps, lhsT=r_(ones_sb),
                                 rhs=r_(bo_sb[:, nh2 * 512:(nh2 + 1) * 512]),
                                 start=False, stop=True)
                nc.vector.tensor_copy(out_sb, ps)
            else:
                nc.vector.tensor_tensor(
                    out=out_sb, in0=ps,
                    in1=bo_bc[:, nh2 * 512:(nh2 + 1) * 512], op=OP.add)
            nc.sync.dma_start(
                out=out_d.ap()[s * ST + t0: s * ST + t0 + 128,
                               nh2 * 512:(nh2 + 1) * 512],
                in_=out_sb)

    def emit_attn_unit(s, u):
        """12 units per supertile: interleavable S/T/O phases of 4 blocks,
        ordered so every T sits a full S-phase behind its scores."""
        seq = [('S', 0), ('S', 1), ('T', 0), ('O', 0), ('S', 2), ('T', 1),
               ('O', 1), ('S', 3), ('T', 2), ('O', 2), ('T', 3), ('O', 3)]
        kind, b = seq[u]
        if kind == 'S':
            S_phase(s, b)
        elif kind == 'T':
            attn_state[('ot', s, b)] = T_phase(s, b)
        else:
            O_phase(s, b, attn_state.pop(('ot', s, b)))

    # ST0 projections run alone (behind the warmup); thereafter supertile
    # s's projection chunks interleave with supertile s-1's 12 attention
    # units. For the last supertile only Q/K interleave there; its V chunks
    # are saved to cover the last supertile's own early attention blocks, so
    # the unpaired engine-jammed tail shrinks from 4 blocks to ~2.
    open_proj(0)
    for idx in range(24):
        emit_proj_unit(0, idx)
    LAST = NST - 1
    for s in range(1, NST):
        open_proj(s)
        n_front = 16 if s == LAST else 24
        counts = [2] * 12
        if s == LAST:
            counts = [2, 2, 1, 1, 2, 1, 1, 2, 1, 1, 1, 1]
        pi = 0
        for u in range(12):
            emit_attn_unit(s - 1, u)
            for _ in range(counts[u]):
                if pi < n_front:
                    emit_proj_unit(s, pi); pi += 1
        while pi < n_front:
            emit_proj_unit(s, pi); pi += 1
    # Final supertile: V-projection chunks (16..23) interleave with the S/T
    # phases, and ALL four O-projections are deferred to the end — a dense,
    # dependency-free pure-PE run that covers the last softmax chains
    # instead of idling behind them. T(3,b) needs v[:, b, :] = V units
    # 16+2b, 17+2b.
    s = LAST
    S_phase(s, 0)
    emit_proj_unit(s, 16); emit_proj_unit(s, 17)
    S_phase(s, 1)
    emit_proj_unit(s, 18); emit_proj_unit(s, 19)
    ot0 = T_phase(s, 0)
    emit_proj_unit(s, 20)
    S_phase(s, 2)
    emit_proj_unit(s, 21)
    ot1 = T_phase(s, 1)
    emit_proj_unit(s, 22)
    S_phase(s, 3)
    emit_proj_unit(s, 23)
    ot2 = T_phase(s, 2)
    O_phase(s, 0, ot0)
    ot3 = T_phase(s, 3)
    O_phase(s, 1, ot1)
    O_phase(s, 2, ot2)
    O_phase(s, 3, ot3)


_NC_CACHE = []


def _get_nc():
    if not _NC_CACHE:
        _NC_CACHE.append(build_bass())
    return _NC_CACHE[0]


def shard_inputs(x, Wq, bq, Wk, bk, Wv, bv, Wo, bo):
    x = np.asarray(x, dtype=np.float32)
    B, S, _ = x.shape
    xf = np.ascontiguousarray(x.reshape(B * S, D))
    assert B * S == N_CORES * TOK

    def warr(W):
        # [m, p, c, n] = W[c*128+p, m*128+n]: every per-m DMA is contiguous.
        W = np.asarray(W, dtype=np.float16)
        return np.ascontiguousarray(
            W.reshape(8, 128, 8, 128).transpose(2, 1, 0, 3).reshape(8, 128, D))

    shared = {
        'wq': warr(Wq),
        'wk': warr(Wk),
        'wv': warr(Wv),
        'wo': warr(Wo),
        'bq': np.ascontiguousarray(np.asarray(bq, dtype=np.float32).reshape(8, 128).T),
        'bk': np.ascontiguousarray(np.asarray(bk, dtype=np.float32).reshape(8, 128).T),
        'bv': np.ascontiguousarray(
            np.broadcast_to(np.asarray(bv, dtype=np.float32).reshape(1, D),
                            (128, D))),
        'bo': np.ascontiguousarray(np.asarray(bo, dtype=np.float16).reshape(1, D)),
        'bobc': np.ascontiguousarray(
            np.broadcast_to(np.asarray(bo, dtype=np.float32).reshape(1, D),
                            (128, D))),
        'ones': np.ones((1, 128), dtype=np.float16),
    }
    in_maps = []
    for c in range(N_CORES):
        xt = np.ascontiguousarray(xf[c * TOK:(c + 1) * TOK, :].T.astype(np.float16))
        in_maps.append({'xt': xt, **shared})
    return (B, S), in_maps


def run(inputs, **spmd_kwargs):
    (B, S), in_maps = shard_inputs(**inputs)
    nc = _get_nc()
    res = run_bass_kernel_spmd(nc, in_maps, list(range(N_CORES)), **spmd_kwargs)
    out = np.concatenate([res.results[c]['out'] for c in range(N_CORES)], axis=0)
    return out.reshape(B, S, D), res


def kernel(x, Wq, bq, Wk, bk, Wv, bv, Wo, bo):
    out, _ = run(dict(x=x, Wq=Wq, bq=bq, Wk=Wk, bk=bk,
                      Wv=Wv, bv=bv, Wo=Wo, bo=bo))
    return out

